# revision 1
# baseline (speedup 1.0000x reference)
"""Conv1D (B=32, L=8192, C_in=64, K=3, F=128, VALID) + bias + ReLU on 8 trn2 cores.

Data-parallel over batch (4 batches per core). Per core:
  - x[b] loads t-major in 512KB chunks ([128, T*64] fp32 tiles, sync ring);
    casts to bf16 split between GpSimd and ScalarE.
  - PE-transposes [128,128] bf16 sub-tiles; both PSUM halves copy out as
    contiguous [64,128] blocks into x_T (DVE). x_T is one manually
    double-buffered [128, 2L] tile: row 64 = ones (bias rides row 64 of the
    k=0 weights), rows 65-127 = zero pad so every matmul is K=128 (measured
    ~1.4x faster than K=64/65); pad rows initialize piecewise with the first
    two batches' fills.
  - out[pos, F]: per 512-position PSUM bank, 4 position-tiles x 3 accumulated
    K=128 matmuls with contiguous x_T windows (strided windows stall the PE);
    one ReLU (ScalarE) per bank into [128, 1024] staging; stores alternate
    between both HWDGE rings. Fill/store emission is interleaved per chunk so
    outputs start streaming early.
"""

import os
import sys

import numpy as np

_TRN_REPO = "/opt/trn_rl_repo"
if _TRN_REPO not in sys.path and os.path.isdir(_TRN_REPO):
    sys.path.insert(0, _TRN_REPO)

import concourse.bass as bass
import concourse.tile as tile
from concourse import bacc, mybir
from concourse.bass_utils import run_bass_kernel_spmd
from concourse.masks import make_identity

B, L, C = 32, 8192, 64
K, F = 3, 128
L_OUT = L - K + 1  # 8190
N_CORES = 8
B_SHARD = B // N_CORES  # 4

MM_DT = mybir.dt.bfloat16

IN_CHUNK = 1024  # positions per input DMA chunk
T_IN = IN_CHUNK // 128  # 16
G = 512  # positions per PSUM output bank
GI = 4  # M-interleave within a bank (G // 128)
OUT_CHUNK = 1024  # positions per output staging tile


def _conv_kernel(tc: tile.TileContext, out_ap, x_ap, w_ap, b_ap, mm_dt):
    nc = tc.nc
    fp32 = mybir.dt.float32

    with (
        tc.tile_pool(name="setup", bufs=1) as setup_pool,
        tc.tile_pool(name="xin", bufs=3) as xin_pool,
        tc.tile_pool(name="xbf", bufs=3) as xbf_pool,
        tc.tile_pool(name="osb", bufs=4) as osb_pool,
        tc.tile_pool(name="pt", bufs=3, space="PSUM") as pt_pool,
        tc.tile_pool(name="po", bufs=5, space="PSUM") as po_pool,
    ):
        # --- one-time setup: weights, bias, identity, xT double-buffer ---
        wstage = setup_pool.tile([C, K * F], fp32)
        for k in range(K):
            nc.scalar.dma_start(out=wstage[:, k * F : (k + 1) * F], in_=w_ap[k])
        bstage = setup_pool.tile([1, F], fp32)
        nc.scalar.dma_start(out=bstage[:, :], in_=b_ap[None, :])

        # w rows 0-63 = w[k]; row 64 of the k=0 slice = bias; rows 65-127
        # zero (pad to K=128 -> measurably faster matmuls).
        wpad = setup_pool.tile([128, K * F], mm_dt)
        nc.vector.memset(wpad[:, :], 0.0)
        nc.vector.tensor_copy(wpad[0:C, :], wstage[:, :])
        nc.vector.tensor_copy(wpad[C : C + 1, 0:F], bstage[:, :])

        ident = setup_pool.tile([128, 128], mm_dt)
        make_identity(nc, ident)

        # xT: manually double-buffered [128, 2*L]; row 64 ones and zero pad
        # rows 65-127 (K=128 matmuls). Pad rows are initialized piecewise
        # alongside the first two batches' fills so they don't gate startup.
        xT = setup_pool.tile([128, 2 * L], mm_dt)

        for b in range(B_SHARD):
            half = (b % 2) * L

            def fill_chunk(b, ci, half):
                c0 = ci * IN_CHUNK
                xin = xin_pool.tile([128, T_IN * C], fp32, name=f"xin_{b}_{ci}", tag="xin")
                if b == 0 and ci == 0:
                    # prime the pipeline: land the first 512 positions early
                    hc = IN_CHUNK // 2
                    for hi in range(2):
                        nc.sync.dma_start(
                            out=xin[:, hi * (hc // 128) * C : (hi + 1) * (hc // 128) * C]
                            .rearrange("p (t c) -> p t c", c=C),
                            in_=x_ap[b, hi * hc : (hi + 1) * hc, :].rearrange(
                                "(t p) c -> p t c", p=128
                            ),
                        )
                else:
                    nc.sync.dma_start(
                        out=xin.rearrange("p (t c) -> p t c", c=C),
                        in_=x_ap[b, c0 : c0 + IN_CHUNK, :].rearrange(
                            "(t p) c -> p t c", p=128
                        ),
                    )
                if b < 2:
                    c0h = half + c0
                    nc.vector.memset(xT[C:128, c0h : c0h + IN_CHUNK], 0.0)
                    nc.gpsimd.memset(xT[C : C + 1, c0h : c0h + IN_CHUNK], 1.0)
                xbf = xbf_pool.tile([128, T_IN * C], mm_dt, name=f"xbf_{b}_{ci}", tag="xbf")
                if b == 0 and ci == 0:
                    hf = T_IN * C // 2
                    nc.scalar.copy(xbf[:, 0:hf], xin[:, 0:hf])
                    nc.gpsimd.tensor_copy(xbf[:, hf:], xin[:, hf:])
                elif ci % 2 == 0:
                    nc.gpsimd.tensor_copy(xbf[:, :], xin[:, :])
                else:
                    nc.scalar.copy(xbf[:, :], xin[:, :])
                # transpose [128,128] sub-tiles; sub-tile j holds t in {2j, 2j+1}
                for j in range(T_IN * C // 128):
                    pt = pt_pool.tile([128, 128], mm_dt, name=f"pt_{b}_{ci}_{j}", tag="pt")
                    nc.tensor.transpose(pt[:, :], xbf[:, j * 128 : (j + 1) * 128], ident)
                    for tt in range(2):
                        q = half + c0 + (2 * j + tt) * 128
                        nc.vector.tensor_copy(
                            xT[0:C, q : q + 128],
                            pt[tt * C : (tt + 1) * C, :],
                        )

            def store_chunk(b, oc, half):
                o0 = oc * OUT_CHUNK
                opos = min(OUT_CHUNK, L_OUT - o0)  # 2048 or 2046
                osb = osb_pool.tile([128, OUT_CHUNK], fp32, name=f"osb_{b}_{oc}", tag="osb")
                n_g = (opos + G - 1) // G
                for gc in range(n_g):
                    g0 = o0 + gc * G
                    gpos = min(G, L_OUT - g0)  # 512 or 510
                    po = po_pool.tile([128, G], fp32, name=f"po_{b}_{oc}_{gc}", tag="po")
                    n_sub = (gpos + 127) // 128
                    for t in range(n_sub):
                        p0 = g0 + t * 128
                        P = min(128, L_OUT - p0)
                        for k in range(K):
                            nc.tensor.matmul(
                                po[0:P, t * F : (t + 1) * F],
                                xT[:, half + p0 + k : half + p0 + k + P],
                                wpad[:, k * F : (k + 1) * F],
                                start=(k == 0),
                                stop=(k == K - 1),
                            )
                    ob = gc * G
                    full_sub = gpos // 128
                    tail_sub = gpos - full_sub * 128
                    if full_sub:
                        nc.scalar.activation(
                            osb[:, ob : ob + full_sub * F],
                            po[:, 0 : full_sub * F],
                            mybir.ActivationFunctionType.Relu,
                        )
                    if tail_sub:
                        nc.scalar.activation(
                            osb[0:tail_sub, ob + full_sub * F : ob + n_sub * F],
                            po[0:tail_sub, full_sub * F : n_sub * F],
                            mybir.ActivationFunctionType.Relu,
                        )
                # store: full tiles in one big DMA, tail tile separately;
                # alternate HWDGE rings so both drain outputs in parallel
                eng = nc.scalar if (b * 4 + oc) % 2 == 0 else nc.sync
                n_full = opos // 128
                tail = opos - n_full * 128
                if n_full:
                    eng.dma_start(
                        out=out_ap[b, o0 : o0 + n_full * 128, :].rearrange(
                            "(t p) f -> p t f", p=128
                        ),
                        in_=osb[:, 0 : n_full * F].rearrange("p (t f) -> p t f", f=F),
                    )
                if tail:
                    eng.dma_start(
                        out=out_ap[b, o0 + n_full * 128 : o0 + opos, :],
                        in_=osb[0:tail, n_full * F : (n_full + 1) * F],
                    )

            n_oc = (L_OUT + OUT_CHUNK - 1) // OUT_CHUNK
            oc_next = 0
            for ci in range(L // IN_CHUNK):
                fill_chunk(b, ci, half)
                filled = (ci + 1) * IN_CHUNK
                while oc_next < n_oc and (oc_next + 1) * OUT_CHUNK + 2 <= filled:
                    store_chunk(b, oc_next, half)
                    oc_next += 1
            while oc_next < n_oc:
                store_chunk(b, oc_next, half)
                oc_next += 1

def build_program(mm_dt=MM_DT):
    nc = bacc.Bacc("TRN2", target_bir_lowering=False, debug=False)
    x = nc.dram_tensor("x", [B_SHARD, L, C], mybir.dt.float32, kind="ExternalInput")
    w = nc.dram_tensor("w", [K, C, F], mybir.dt.float32, kind="ExternalInput")
    bb = nc.dram_tensor("b", [F], mybir.dt.float32, kind="ExternalInput")
    out = nc.dram_tensor(
        "out", [B_SHARD, L_OUT, F], mybir.dt.float32, kind="ExternalOutput"
    )
    with tile.TileContext(nc) as tc:
        _conv_kernel(tc, out.ap(), x.ap(), w.ap(), bb.ap(), mm_dt)
    nc.compile()
    return nc


def kernel(x, w, b, _trace=False, _trace_kwargs=None):
    x = np.ascontiguousarray(np.asarray(x, dtype=np.float32))
    w = np.ascontiguousarray(np.asarray(w, dtype=np.float32))
    b = np.ascontiguousarray(np.asarray(b, dtype=np.float32))
    assert x.shape == (B, L, C) and w.shape == (K, C, F) and b.shape == (F,)

    nc = build_program()
    in_maps = [
        {"x": x[i * B_SHARD : (i + 1) * B_SHARD], "w": w, "b": b}
        for i in range(N_CORES)
    ]
    res = run_bass_kernel_spmd(
        nc,
        in_maps,
        core_ids=list(range(N_CORES)),
        trace=_trace,
        **(_trace_kwargs or {}),
    )
    out = np.concatenate([r["out"] for r in res.results], axis=0)
    if _trace:
        return out, res
    return out


if __name__ == "__main__":
    rng = np.random.default_rng(0)
    x = rng.standard_normal((B, L, C), dtype=np.float32)
    w = rng.standard_normal((K, C, F), dtype=np.float32) * 0.08
    b = np.zeros((F,), dtype=np.float32)
    out = kernel(x, w, b)
    print("out", out.shape, out.dtype, float(np.abs(out).max()))



# revision 18
# speedup vs baseline: 1.1627x; 1.1627x over previous
"""Conv1D (B=32, L=8192, C_in=64, K=3, F=128, VALID) + bias + ReLU on 8 trn2 cores.

Data-parallel over batch (4 batches per core). Pairs-polyphase design:

  - Input loads 2 consecutive positions per partition (512B descriptors, the
    DMA sweet spot: <512B descriptors pay a 2x latency multiplier) via
    gpsimd swDGE DMAs that cast fp32->bf16 in flight (no compute-cast).
  - PE-transpose of each [128, (s,c)] sub-tile yields even-phase channels in
    rows 0:64 and odd-phase in rows 64:128; two [64,1024] DVE copies per
    chunk build xP where column j = [x[2j] channels; x[2j+1] channels].
  - Polyphase matmuls, 2 per output phase per 128-pair tile (all c=128;
    c=64 matmuls hard-crash the device, so the single-tap matmuls pad the
    weight tile with zero rows instead):
      out[2j]   = xP[:,j]@[w0;w1] + xP[:,j+1]@[w2;0]
      out[2j+1] = xP[:,j]@[0;w0]  + xP[:,j+1]@[w1;w2]
  - PSUM bank [128,512] = 2 pair-tiles x (even F || odd F); ReLU (split
    scalar/DVE) writes bf16 staging; partition p holds positions 2p,2p+1 so
    stores are 512B descriptors; bf16 output halves write bytes (host
    upcasts to fp32; rel err ~4e-3 << 2e-2 budget).
"""

import os
import sys

import numpy as np

_TRN_REPO = "/opt/trn_rl_repo"
if _TRN_REPO not in sys.path and os.path.isdir(_TRN_REPO):
    sys.path.insert(0, _TRN_REPO)

import concourse.bass as bass
import concourse.tile as tile
from concourse import bacc, mybir
from concourse.bass_utils import run_bass_kernel_spmd
from concourse.masks import make_identity

B, L, C = 32, 8192, 64
K, F = 3, 128
L_OUT = L - K + 1  # 8190
N_CORES = 8
B_SHARD = B // N_CORES  # 4

MM_DT = mybir.dt.bfloat16
OUT_DT = mybir.dt.bfloat16

USE_SWDGE = os.environ.get("KERNEL_SWDGE", "1") == "1"

IN_CHUNK = 2048  # positions per input DMA chunk (1024 pairs)
N_CI = L // IN_CHUNK  # 4
T_IN = IN_CHUNK // 256  # 8 transposes per chunk
PAIRS = L // 2  # 4096 pairs per batch (pair 4095 is compute-garbage)
XP_PAD = 8  # zeroed pad cols so the last +1 window stays in bounds
XP_W = PAIRS + XP_PAD  # 4104
BANK_PAIRS = 256  # pairs per PSUM bank (2 pair-tiles of 128)
ST_PAIRS = 1024  # pairs per output staging tile (4 banks)
N_ST = PAIRS // ST_PAIRS  # 4 stores per batch


def _conv_kernel(tc: tile.TileContext, out_ap, x_ap, w_ap, b_ap, has_bias):
    nc = tc.nc
    fp32 = mybir.dt.float32

    with (
        tc.tile_pool(name="setup", bufs=1) as setup_pool,
        tc.tile_pool(name="xin", bufs=3) as xin_pool,
        tc.tile_pool(name="xbf", bufs=3) as xbf_pool,
        tc.tile_pool(name="osb", bufs=3) as osb_pool,
        tc.tile_pool(name="pt", bufs=3, space="PSUM") as pt_pool,
        tc.tile_pool(name="po", bufs=4, space="PSUM") as po_pool,
    ):
        # --- one-time setup ---
        wstage = setup_pool.tile([C, K * F], fp32)
        for k in range(K):
            nc.scalar.dma_start(out=wstage[:, k * F : (k + 1) * F], in_=w_ap[k])

        # wAll cols: [0:F]=[w0;w1]  [F:2F]=[w1;w2]  [2F:3F]=[w2;0]  [3F:4F]=[0;w0]
        wAll = setup_pool.tile([128, 4 * F], MM_DT)
        nc.vector.memset(wAll[:, :], 0.0)
        nc.vector.tensor_copy(wAll[0:C, 0:F], wstage[:, 0:F])
        nc.vector.tensor_copy(wAll[C:128, 0:F], wstage[:, F : 2 * F])
        nc.vector.tensor_copy(wAll[0:C, F : 2 * F], wstage[:, F : 2 * F])
        nc.vector.tensor_copy(wAll[C:128, F : 2 * F], wstage[:, 2 * F : 3 * F])
        nc.vector.tensor_copy(wAll[0:C, 2 * F : 3 * F], wstage[:, 2 * F : 3 * F])
        nc.vector.tensor_copy(wAll[C:128, 3 * F : 4 * F], wstage[:, 0:F])

        if has_bias:
            bstage = setup_pool.tile([1, F], fp32)
            nc.scalar.dma_start(out=bstage[:, :], in_=b_ap[None, :])
            ones = setup_pool.tile([1, 128], MM_DT)
            nc.vector.memset(ones[:, :], 1.0)
            brow = setup_pool.tile([1, F], MM_DT)
            nc.vector.tensor_copy(brow[:, :], bstage[:, :])

        ident = setup_pool.tile([128, 128], MM_DT)
        make_identity(nc, ident)

        # xP: manually double-buffered [128, 2*XP_W] packed pair array.
        xP = setup_pool.tile([128, 2 * XP_W], MM_DT)
        for h in range(2):
            nc.vector.memset(xP[:, h * XP_W + PAIRS : (h + 1) * XP_W], 0.0)

        relu_ctr = [0]
        store_ctr = [0]

        for b in range(B_SHARD):
            xoff = (b % 2) * XP_W

            def fill_chunk(b, ci, xoff):
                c0 = ci * IN_CHUNK
                if USE_SWDGE:
                    # swDGE cast-DMA: HBM fp32 pairs -> SBUF bf16, 512B desc
                    xin = xin_pool.tile(
                        [128, 8 * 128], MM_DT, name=f"xin_{b}_{ci}", tag="xin"
                    )
                    nc.gpsimd.dma_start(
                        out=xin.rearrange("p (t s c) -> p t s c", s=2, c=C),
                        in_=x_ap[b, c0 : c0 + IN_CHUNK, :].rearrange(
                            "(t p s) c -> p t s c", p=128, s=2
                        ),
                    )
                else:
                    xf = xin_pool.tile(
                        [128, 8 * 128], fp32, name=f"xf_{b}_{ci}", tag="xf"
                    )
                    nc.sync.dma_start(
                        out=xf.rearrange("p (t s c) -> p t s c", s=2, c=C),
                        in_=x_ap[b, c0 : c0 + IN_CHUNK, :].rearrange(
                            "(t p s) c -> p t s c", p=128, s=2
                        ),
                    )
                    xin = xbf_pool.tile(
                        [128, 8 * 128], MM_DT, name=f"xin_{b}_{ci}", tag="xin"
                    )
                    if ci % 2 == 0:
                        nc.scalar.copy(xin[:, :], xf[:, :])
                    else:
                        nc.gpsimd.tensor_copy(xin[:, :], xf[:, :])

                pt8 = pt_pool.tile([128, 1024], MM_DT, name=f"pt8_{b}_{ci}", tag="pt8")
                for j in range(T_IN):
                    nc.tensor.transpose(
                        pt8[:, j * 128 : (j + 1) * 128],
                        xin[:, j * 128 : (j + 1) * 128],
                        ident,
                    )
                q = xoff + ci * (IN_CHUNK // 2)
                nc.vector.tensor_copy(xP[0:C, q : q + 1024], pt8[0:C, :])
                nc.vector.tensor_copy(xP[C:128, q : q + 1024], pt8[C:128, :])

            def store_chunk(b, s, xoff):
                o0 = s * 2 * ST_PAIRS
                osb = osb_pool.tile(
                    [128, 2 * ST_PAIRS], OUT_DT, name=f"osb_{b}_{s}", tag="osb"
                )
                for bk in range(4):
                    po = po_pool.tile(
                        [128, 512], fp32, name=f"po_{b}_{s}_{bk}", tag="po"
                    )
                    for pt in range(2):
                        j0 = xoff + s * ST_PAIRS + bk * BANK_PAIRS + pt * 128
                        c = pt * 256
                        # even: out[2j] = x[2j]w0 + x[2j+1]w1 + x[2j+2]w2
                        nc.tensor.matmul(
                            po[:, c : c + 128],
                            xP[:, j0 : j0 + 128],
                            wAll[:, 0:F],
                            start=True,
                            stop=False,
                        )
                        nc.tensor.matmul(
                            po[:, c : c + 128],
                            xP[:, j0 + 1 : j0 + 129],
                            wAll[:, 2 * F : 3 * F],
                            start=False,
                            stop=not has_bias,
                        )
                        if has_bias:
                            nc.tensor.matmul(
                                po[:, c : c + 128],
                                ones[:, :],
                                brow[:, :],
                                start=False,
                                stop=True,
                            )
                        # odd: out[2j+1] = x[2j+1]w0 + x[2j+2]w1 + x[2j+3]w2
                        nc.tensor.matmul(
                            po[:, c + 128 : c + 256],
                            xP[:, j0 : j0 + 128],
                            wAll[:, 3 * F : 4 * F],
                            start=True,
                            stop=False,
                        )
                        nc.tensor.matmul(
                            po[:, c + 128 : c + 256],
                            xP[:, j0 + 1 : j0 + 129],
                            wAll[:, F : 2 * F],
                            start=False,
                            stop=not has_bias,
                        )
                        if has_bias:
                            nc.tensor.matmul(
                                po[:, c + 128 : c + 256],
                                ones[:, :],
                                brow[:, :],
                                start=False,
                                stop=True,
                            )
                    ob = bk * 512
                    # ReLU + fp32->bf16, split 5:3 scalar:DVE
                    if relu_ctr[0] % 8 < 5:
                        nc.scalar.activation(
                            osb[:, ob : ob + 512],
                            po[:, :],
                            mybir.ActivationFunctionType.Relu,
                        )
                    else:
                        nc.vector.tensor_scalar_max(
                            osb[:, ob : ob + 512], po[:, :], 0.0
                        )
                    relu_ctr[0] += 1
                # stores: 512B descriptors (pair of positions), alternate rings
                eng = nc.scalar if store_ctr[0] % 2 == 0 else nc.sync
                store_ctr[0] += 1
                if s < N_ST - 1:
                    eng.dma_start(
                        out=out_ap[b, o0 : o0 + 2048, :].rearrange(
                            "(g p s2) f -> p g s2 f", p=128, s2=2
                        ),
                        in_=osb.rearrange("p (g s2 f) -> p g s2 f", s2=2, f=F),
                    )
                else:
                    # last store: drop garbage pair 4095 (positions 8190/8191)
                    eng.dma_start(
                        out=out_ap[b, o0 : o0 + 1792, :].rearrange(
                            "(g p s2) f -> p g s2 f", p=128, s2=2
                        ),
                        in_=osb[:, 0:1792].rearrange(
                            "p (g s2 f) -> p g s2 f", s2=2, f=F
                        ),
                    )
                    eng.dma_start(
                        out=out_ap[b, o0 + 1792 : o0 + 2046, :].rearrange(
                            "(p s2) f -> p s2 f", p=127, s2=2
                        ),
                        in_=osb[0:127, 1792:2048].rearrange(
                            "p (s2 f) -> p s2 f", s2=2
                        ),
                    )

            # store s needs xP cols through 1024(s+1) (the +1 window crosses
            # one col into chunk s+1), so store s follows fill s+1.
            fill_chunk(b, 0, xoff)
            fill_chunk(b, 1, xoff)
            store_chunk(b, 0, xoff)
            fill_chunk(b, 2, xoff)
            store_chunk(b, 1, xoff)
            fill_chunk(b, 3, xoff)
            store_chunk(b, 2, xoff)
            store_chunk(b, 3, xoff)


def build_program(has_bias):
    nc = bacc.Bacc("TRN2", target_bir_lowering=False, debug=False)
    x = nc.dram_tensor("x", [B_SHARD, L, C], mybir.dt.float32, kind="ExternalInput")
    w = nc.dram_tensor("w", [K, C, F], mybir.dt.float32, kind="ExternalInput")
    bb = nc.dram_tensor("b", [F], mybir.dt.float32, kind="ExternalInput")
    out = nc.dram_tensor(
        "out", [B_SHARD, L_OUT, F], OUT_DT, kind="ExternalOutput"
    )
    with tile.TileContext(nc) as tc:
        _conv_kernel(tc, out.ap(), x.ap(), w.ap(), bb.ap(), has_bias)
    nc.compile()
    return nc


def kernel(x, w, b, _trace=False, _trace_kwargs=None):
    x = np.ascontiguousarray(np.asarray(x, dtype=np.float32))
    w = np.ascontiguousarray(np.asarray(w, dtype=np.float32))
    b = np.ascontiguousarray(np.asarray(b, dtype=np.float32))
    assert x.shape == (B, L, C) and w.shape == (K, C, F) and b.shape == (F,)

    nc = build_program(has_bias=bool(np.any(b)))
    in_maps = [
        {"x": x[i * B_SHARD : (i + 1) * B_SHARD], "w": w, "b": b}
        for i in range(N_CORES)
    ]
    res = run_bass_kernel_spmd(
        nc,
        in_maps,
        core_ids=list(range(N_CORES)),
        trace=_trace,
        **(_trace_kwargs or {}),
    )
    out = np.concatenate(
        [np.asarray(r["out"]).astype(np.float32) for r in res.results], axis=0
    )
    if _trace:
        return out, res
    return out


if __name__ == "__main__":
    rng = np.random.default_rng(0)
    x = rng.standard_normal((B, L, C), dtype=np.float32)
    w = rng.standard_normal((K, C, F), dtype=np.float32) * 0.08
    b = np.zeros((F,), dtype=np.float32)
    out = kernel(x, w, b)
    print("out", out.shape, out.dtype, float(np.abs(out).max()))


# revision 21
# speedup vs baseline: 1.2765x; 1.0979x over previous
"""Conv1D (B=32, L=8192, C_in=64, K=3, F=128, VALID) + bias + ReLU on 8 trn2 cores.

Data-parallel over batch (4 batches per core). Pairs-polyphase design:

  - Input loads 2 consecutive positions per partition (512B descriptors, the
    DMA sweet spot: <512B descriptors pay a 2x latency multiplier) via
    gpsimd swDGE DMAs that cast fp32->bf16 in flight (no compute-cast).
  - PE-transpose of each [128, (s,c)] sub-tile yields even-phase channels in
    rows 0:64 and odd-phase in rows 64:128; two [64,1024] DVE copies per
    chunk build xP where column j = [x[2j] channels; x[2j+1] channels].
  - Polyphase matmuls, 2 per output phase per 128-pair tile (all c=128;
    c=64 matmuls hard-crash the device, so the single-tap matmuls pad the
    weight tile with zero rows instead):
      out[2j]   = xP[:,j]@[w0;w1] + xP[:,j+1]@[w2;0]
      out[2j+1] = xP[:,j]@[0;w0]  + xP[:,j+1]@[w1;w2]
  - PSUM bank [128,512] = 2 pair-tiles x (even F || odd F); ReLU (split
    scalar/DVE) writes bf16 staging; partition p holds positions 2p,2p+1 so
    stores are 512B descriptors; bf16 output halves write bytes (host
    upcasts to fp32; rel err ~4e-3 << 2e-2 budget).
"""

import os
import sys

import numpy as np

_TRN_REPO = "/opt/trn_rl_repo"
if _TRN_REPO not in sys.path and os.path.isdir(_TRN_REPO):
    sys.path.insert(0, _TRN_REPO)

import concourse.bass as bass
import concourse.tile as tile
from concourse import bacc, mybir
from concourse.bass_utils import run_bass_kernel_spmd
from concourse.masks import make_identity

B, L, C = 32, 8192, 64
K, F = 3, 128
L_OUT = L - K + 1  # 8190
N_CORES = 8
B_SHARD = B // N_CORES  # 4

MM_DT = mybir.dt.bfloat16
OUT_DT = mybir.dt.bfloat16

# swDGE cast-DMA measured 41.9ns per 256B descriptor vs hwDGE 28.9ns per
# 512B — hwDGE fp32 loads + compute casts are cheaper on the DMA engines.
USE_SWDGE = os.environ.get("KERNEL_SWDGE", "0") == "1"

IN_CHUNK = 2048  # positions per input DMA chunk (1024 pairs)
N_CI = L // IN_CHUNK  # 4
T_IN = IN_CHUNK // 256  # 8 transposes per chunk
PAIRS = L // 2  # 4096 pairs per batch (pair 4095 is compute-garbage)
XP_PAD = 8  # zeroed pad cols so the last +1 window stays in bounds
XP_W = PAIRS + XP_PAD  # 4104
BANK_PAIRS = 256  # pairs per PSUM bank (2 pair-tiles of 128)
ST_PAIRS = 1024  # pairs per output staging tile (4 banks)
N_ST = PAIRS // ST_PAIRS  # 4 stores per batch


def _conv_kernel(tc: tile.TileContext, out_ap, x_ap, w_ap, b_ap, has_bias):
    nc = tc.nc
    fp32 = mybir.dt.float32

    with (
        tc.tile_pool(name="setup", bufs=1) as setup_pool,
        tc.tile_pool(name="xin", bufs=3) as xin_pool,
        tc.tile_pool(name="xbf", bufs=3) as xbf_pool,
        tc.tile_pool(name="osb", bufs=3) as osb_pool,
        tc.tile_pool(name="pt", bufs=3, space="PSUM") as pt_pool,
        tc.tile_pool(name="po", bufs=4, space="PSUM") as po_pool,
    ):
        # --- one-time setup ---
        wstage = setup_pool.tile([C, K * F], fp32)
        for k in range(K):
            nc.scalar.dma_start(out=wstage[:, k * F : (k + 1) * F], in_=w_ap[k])

        # wAll cols: [0:F]=[w0;w1]  [F:2F]=[w1;w2]  [2F:3F]=[w2;0]  [3F:4F]=[0;w0]
        wAll = setup_pool.tile([128, 4 * F], MM_DT)
        nc.vector.memset(wAll[:, :], 0.0)
        nc.vector.tensor_copy(wAll[0:C, 0:F], wstage[:, 0:F])
        nc.vector.tensor_copy(wAll[C:128, 0:F], wstage[:, F : 2 * F])
        nc.vector.tensor_copy(wAll[0:C, F : 2 * F], wstage[:, F : 2 * F])
        nc.vector.tensor_copy(wAll[C:128, F : 2 * F], wstage[:, 2 * F : 3 * F])
        nc.vector.tensor_copy(wAll[0:C, 2 * F : 3 * F], wstage[:, 2 * F : 3 * F])
        nc.vector.tensor_copy(wAll[C:128, 3 * F : 4 * F], wstage[:, 0:F])

        if has_bias:
            bstage = setup_pool.tile([1, F], fp32)
            nc.scalar.dma_start(out=bstage[:, :], in_=b_ap[None, :])
            ones = setup_pool.tile([1, 128], MM_DT)
            nc.vector.memset(ones[:, :], 1.0)
            brow = setup_pool.tile([1, F], MM_DT)
            nc.vector.tensor_copy(brow[:, :], bstage[:, :])

        ident = setup_pool.tile([128, 128], MM_DT)
        make_identity(nc, ident)

        # xP: manually double-buffered [128, 2*XP_W] packed pair array.
        xP = setup_pool.tile([128, 2 * XP_W], MM_DT)
        for h in range(2):
            nc.vector.memset(xP[:, h * XP_W + PAIRS : (h + 1) * XP_W], 0.0)

        relu_ctr = [0]
        store_ctr = [0]
        cast_ctr = [0]
        # cast engine rotation: gpsimd is slow (~3.7us/tile) so it gets few
        CAST_PAT = ["g", "s", "d", "d", "g", "s", "d", "s"]

        for b in range(B_SHARD):
            xoff = (b % 2) * XP_W

            def fill_chunk(b, ci, xoff):
                c0 = ci * IN_CHUNK
                if USE_SWDGE:
                    # swDGE cast-DMA: HBM fp32 pairs -> SBUF bf16, 512B desc
                    xin = xin_pool.tile(
                        [128, 8 * 128], MM_DT, name=f"xin_{b}_{ci}", tag="xin"
                    )
                    nc.gpsimd.dma_start(
                        out=xin.rearrange("p (t s c) -> p t s c", s=2, c=C),
                        in_=x_ap[b, c0 : c0 + IN_CHUNK, :].rearrange(
                            "(t p s) c -> p t s c", p=128, s=2
                        ),
                    )
                else:
                    xf = xin_pool.tile(
                        [128, 8 * 128], fp32, name=f"xf_{b}_{ci}", tag="xf"
                    )
                    nc.sync.dma_start(
                        out=xf.rearrange("p (t s c) -> p t s c", s=2, c=C),
                        in_=x_ap[b, c0 : c0 + IN_CHUNK, :].rearrange(
                            "(t p s) c -> p t s c", p=128, s=2
                        ),
                    )
                    xin = xbf_pool.tile(
                        [128, 8 * 128], MM_DT, name=f"xin_{b}_{ci}", tag="xin"
                    )
                    ce = CAST_PAT[cast_ctr[0] % len(CAST_PAT)]
                    cast_ctr[0] += 1
                    if ce == "g":
                        nc.gpsimd.tensor_copy(xin[:, :], xf[:, :])
                    elif ce == "s":
                        nc.scalar.copy(xin[:, :], xf[:, :])
                    else:
                        nc.vector.tensor_copy(xin[:, :], xf[:, :])

                pt8 = pt_pool.tile([128, 1024], MM_DT, name=f"pt8_{b}_{ci}", tag="pt8")
                for j in range(T_IN):
                    nc.tensor.transpose(
                        pt8[:, j * 128 : (j + 1) * 128],
                        xin[:, j * 128 : (j + 1) * 128],
                        ident,
                    )
                q = xoff + ci * (IN_CHUNK // 2)
                nc.vector.tensor_copy(xP[0:C, q : q + 1024], pt8[0:C, :])
                nc.vector.tensor_copy(xP[C:128, q : q + 1024], pt8[C:128, :])

            def store_chunk(b, s, xoff):
                o0 = s * 2 * ST_PAIRS
                osb = osb_pool.tile(
                    [128, 2 * ST_PAIRS], OUT_DT, name=f"osb_{b}_{s}", tag="osb"
                )
                for bk in range(4):
                    po = po_pool.tile(
                        [128, 512], fp32, name=f"po_{b}_{s}_{bk}", tag="po"
                    )
                    for pt in range(2):
                        j0 = xoff + s * ST_PAIRS + bk * BANK_PAIRS + pt * 128
                        c = pt * 256
                        # even: out[2j] = x[2j]w0 + x[2j+1]w1 + x[2j+2]w2
                        nc.tensor.matmul(
                            po[:, c : c + 128],
                            xP[:, j0 : j0 + 128],
                            wAll[:, 0:F],
                            start=True,
                            stop=False,
                        )
                        nc.tensor.matmul(
                            po[:, c : c + 128],
                            xP[:, j0 + 1 : j0 + 129],
                            wAll[:, 2 * F : 3 * F],
                            start=False,
                            stop=not has_bias,
                        )
                        if has_bias:
                            nc.tensor.matmul(
                                po[:, c : c + 128],
                                ones[:, :],
                                brow[:, :],
                                start=False,
                                stop=True,
                            )
                        # odd: out[2j+1] = x[2j+1]w0 + x[2j+2]w1 + x[2j+3]w2
                        nc.tensor.matmul(
                            po[:, c + 128 : c + 256],
                            xP[:, j0 : j0 + 128],
                            wAll[:, 3 * F : 4 * F],
                            start=True,
                            stop=False,
                        )
                        nc.tensor.matmul(
                            po[:, c + 128 : c + 256],
                            xP[:, j0 + 1 : j0 + 129],
                            wAll[:, F : 2 * F],
                            start=False,
                            stop=not has_bias,
                        )
                        if has_bias:
                            nc.tensor.matmul(
                                po[:, c + 128 : c + 256],
                                ones[:, :],
                                brow[:, :],
                                start=False,
                                stop=True,
                            )
                    ob = bk * 512
                    # ReLU + fp32->bf16, split 5:3 scalar:DVE
                    if relu_ctr[0] % 8 < 5:
                        nc.scalar.activation(
                            osb[:, ob : ob + 512],
                            po[:, :],
                            mybir.ActivationFunctionType.Relu,
                        )
                    else:
                        nc.vector.tensor_scalar_max(
                            osb[:, ob : ob + 512], po[:, :], 0.0
                        )
                    relu_ctr[0] += 1
                # stores: 512B descriptors (pair of positions), alternate rings
                eng = nc.scalar if store_ctr[0] % 2 == 0 else nc.sync
                store_ctr[0] += 1
                if s < N_ST - 1:
                    eng.dma_start(
                        out=out_ap[b, o0 : o0 + 2048, :].rearrange(
                            "(g p s2) f -> p g s2 f", p=128, s2=2
                        ),
                        in_=osb.rearrange("p (g s2 f) -> p g s2 f", s2=2, f=F),
                    )
                else:
                    # last store: drop garbage pair 4095 (positions 8190/8191)
                    eng.dma_start(
                        out=out_ap[b, o0 : o0 + 1792, :].rearrange(
                            "(g p s2) f -> p g s2 f", p=128, s2=2
                        ),
                        in_=osb[:, 0:1792].rearrange(
                            "p (g s2 f) -> p g s2 f", s2=2, f=F
                        ),
                    )
                    eng.dma_start(
                        out=out_ap[b, o0 + 1792 : o0 + 2046, :].rearrange(
                            "(p s2) f -> p s2 f", p=127, s2=2
                        ),
                        in_=osb[0:127, 1792:2048].rearrange(
                            "p (s2 f) -> p s2 f", s2=2
                        ),
                    )

            # store s needs xP cols through 1024(s+1) (the +1 window crosses
            # one col into chunk s+1), so store s follows fill s+1.
            fill_chunk(b, 0, xoff)
            fill_chunk(b, 1, xoff)
            store_chunk(b, 0, xoff)
            fill_chunk(b, 2, xoff)
            store_chunk(b, 1, xoff)
            fill_chunk(b, 3, xoff)
            store_chunk(b, 2, xoff)
            store_chunk(b, 3, xoff)


def build_program(has_bias):
    nc = bacc.Bacc("TRN2", target_bir_lowering=False, debug=False)
    x = nc.dram_tensor("x", [B_SHARD, L, C], mybir.dt.float32, kind="ExternalInput")
    w = nc.dram_tensor("w", [K, C, F], mybir.dt.float32, kind="ExternalInput")
    bb = nc.dram_tensor("b", [F], mybir.dt.float32, kind="ExternalInput")
    out = nc.dram_tensor(
        "out", [B_SHARD, L_OUT, F], OUT_DT, kind="ExternalOutput"
    )
    with tile.TileContext(nc) as tc:
        _conv_kernel(tc, out.ap(), x.ap(), w.ap(), bb.ap(), has_bias)
    nc.compile()
    return nc


def kernel(x, w, b, _trace=False, _trace_kwargs=None):
    x = np.ascontiguousarray(np.asarray(x, dtype=np.float32))
    w = np.ascontiguousarray(np.asarray(w, dtype=np.float32))
    b = np.ascontiguousarray(np.asarray(b, dtype=np.float32))
    assert x.shape == (B, L, C) and w.shape == (K, C, F) and b.shape == (F,)

    nc = build_program(has_bias=bool(np.any(b)))
    in_maps = [
        {"x": x[i * B_SHARD : (i + 1) * B_SHARD], "w": w, "b": b}
        for i in range(N_CORES)
    ]
    res = run_bass_kernel_spmd(
        nc,
        in_maps,
        core_ids=list(range(N_CORES)),
        trace=_trace,
        **(_trace_kwargs or {}),
    )
    out = np.concatenate(
        [np.asarray(r["out"]).astype(np.float32) for r in res.results], axis=0
    )
    if _trace:
        return out, res
    return out


if __name__ == "__main__":
    rng = np.random.default_rng(0)
    x = rng.standard_normal((B, L, C), dtype=np.float32)
    w = rng.standard_normal((K, C, F), dtype=np.float32) * 0.08
    b = np.zeros((F,), dtype=np.float32)
    out = kernel(x, w, b)
    print("out", out.shape, out.dtype, float(np.abs(out).max()))


# revision 26
# speedup vs baseline: 1.3230x; 1.0364x over previous
"""Conv1D (B=32, L=8192, C_in=64, K=3, F=128, VALID) + bias + ReLU on 8 trn2 cores.

Data-parallel over batch (4 batches per core). Pairs-polyphase design:

  - Input loads 2 consecutive positions per partition (512B descriptors, the
    DMA sweet spot: <512B descriptors pay a 2x latency multiplier) via
    gpsimd swDGE DMAs that cast fp32->bf16 in flight (no compute-cast).
  - PE-transpose of each [128, (s,c)] sub-tile yields even-phase channels in
    rows 0:64 and odd-phase in rows 64:128; two [64,1024] DVE copies per
    chunk build xP where column j = [x[2j] channels; x[2j+1] channels].
  - Polyphase matmuls, 2 per output phase per 128-pair tile (all c=128;
    c=64 matmuls hard-crash the device, so the single-tap matmuls pad the
    weight tile with zero rows instead):
      out[2j]   = xP[:,j]@[w0;w1] + xP[:,j+1]@[w2;0]
      out[2j+1] = xP[:,j]@[0;w0]  + xP[:,j+1]@[w1;w2]
  - PSUM bank [128,512] = 2 pair-tiles x (even F || odd F); ReLU (split
    scalar/DVE) writes bf16 staging; partition p holds positions 2p,2p+1 so
    stores are 512B descriptors; bf16 output halves write bytes (host
    upcasts to fp32; rel err ~4e-3 << 2e-2 budget).
"""

import os
import sys

import numpy as np

_TRN_REPO = "/opt/trn_rl_repo"
if _TRN_REPO not in sys.path and os.path.isdir(_TRN_REPO):
    sys.path.insert(0, _TRN_REPO)

import concourse.bass as bass
import concourse.tile as tile
from concourse import bacc, mybir
from concourse.bass_utils import run_bass_kernel_spmd
from concourse.masks import make_identity

B, L, C = 32, 8192, 64
K, F = 3, 128
L_OUT = L - K + 1  # 8190
N_CORES = 8
B_SHARD = B // N_CORES  # 4

MM_DT = mybir.dt.bfloat16
OUT_DT = mybir.dt.bfloat16

# swDGE cast-DMA measured 41.9ns per 256B descriptor vs hwDGE 28.9ns per
# 512B — hwDGE fp32 loads + compute casts are cheaper on the DMA engines.
USE_SWDGE = os.environ.get("KERNEL_SWDGE", "0") == "1"

IN_CHUNK = 2048  # positions per input DMA chunk (1024 pairs)
N_CI = L // IN_CHUNK  # 4
T_IN = IN_CHUNK // 256  # 8 transposes per chunk
PAIRS = L // 2  # 4096 pairs per batch (pair 4095 is compute-garbage)
XP_PAD = 8  # zeroed pad cols so the last +1 window stays in bounds
XP_W = PAIRS + XP_PAD  # 4104
BANK_PAIRS = 256  # pairs per PSUM bank (2 pair-tiles of 128)
ST_PAIRS = 1024  # pairs per output staging tile (4 banks)
N_ST = PAIRS // ST_PAIRS  # 4 stores per batch


def _conv_kernel(tc: tile.TileContext, out_ap, x_ap, w_ap, b_ap, has_bias):
    nc = tc.nc
    fp32 = mybir.dt.float32

    with (
        tc.tile_pool(name="setup", bufs=1) as setup_pool,
        tc.tile_pool(name="xin", bufs=3) as xin_pool,
        tc.tile_pool(name="xbf", bufs=3) as xbf_pool,
        tc.tile_pool(name="osb", bufs=3) as osb_pool,
        tc.tile_pool(name="pt", bufs=3, space="PSUM") as pt_pool,
        tc.tile_pool(name="po", bufs=4, space="PSUM") as po_pool,
    ):
        # --- one-time setup ---
        wstage = setup_pool.tile([C, K * F], fp32)
        for k in range(K):
            nc.scalar.dma_start(out=wstage[:, k * F : (k + 1) * F], in_=w_ap[k])

        # wAll cols: [0:F]=[w0;w1]  [F:2F]=[w1;w2]  [2F:3F]=[w2;0]  [3F:4F]=[0;w0]
        wAll = setup_pool.tile([128, 4 * F], MM_DT)
        nc.vector.memset(wAll[:, :], 0.0)
        nc.vector.tensor_copy(wAll[0:C, 0:F], wstage[:, 0:F])
        nc.vector.tensor_copy(wAll[C:128, 0:F], wstage[:, F : 2 * F])
        nc.vector.tensor_copy(wAll[0:C, F : 2 * F], wstage[:, F : 2 * F])
        nc.vector.tensor_copy(wAll[C:128, F : 2 * F], wstage[:, 2 * F : 3 * F])
        nc.vector.tensor_copy(wAll[0:C, 2 * F : 3 * F], wstage[:, 2 * F : 3 * F])
        nc.vector.tensor_copy(wAll[C:128, 3 * F : 4 * F], wstage[:, 0:F])

        if has_bias:
            bstage = setup_pool.tile([1, F], fp32)
            nc.scalar.dma_start(out=bstage[:, :], in_=b_ap[None, :])
            ones = setup_pool.tile([1, 128], MM_DT)
            nc.vector.memset(ones[:, :], 1.0)
            brow = setup_pool.tile([1, F], MM_DT)
            nc.vector.tensor_copy(brow[:, :], bstage[:, :])

        ident = setup_pool.tile([128, 128], MM_DT)
        make_identity(nc, ident)

        # xP: manually double-buffered [128, 2*XP_W] packed pair array.
        xP = setup_pool.tile([128, 2 * XP_W], MM_DT)
        for h in range(2):
            nc.vector.memset(xP[:, h * XP_W + PAIRS : (h + 1) * XP_W], 0.0)

        relu_ctr = [0]
        store_ctr = [0]
        cast_ctr = [0]
        # cast engine rotation: gpsimd excluded — its ~3.6us casts stall the
        # whole downstream pipeline (observed as all-engines-idle windows)
        CAST_PAT = ["s", "d"]

        if True:

            def fill_chunk(b, ci):
                xoff = (b % 2) * XP_W
                c0 = ci * IN_CHUNK
                if USE_SWDGE:
                    # swDGE cast-DMA: HBM fp32 pairs -> SBUF bf16, 512B desc
                    xin = xin_pool.tile(
                        [128, 8 * 128], MM_DT, name=f"xin_{b}_{ci}", tag="xin"
                    )
                    nc.gpsimd.dma_start(
                        out=xin.rearrange("p (t s c) -> p t s c", s=2, c=C),
                        in_=x_ap[b, c0 : c0 + IN_CHUNK, :].rearrange(
                            "(t p s) c -> p t s c", p=128, s=2
                        ),
                    )
                else:
                    xf = xin_pool.tile(
                        [128, 8 * 128], fp32, name=f"xf_{b}_{ci}", tag="xf"
                    )
                    nc.sync.dma_start(
                        out=xf.rearrange("p (t s c) -> p t s c", s=2, c=C),
                        in_=x_ap[b, c0 : c0 + IN_CHUNK, :].rearrange(
                            "(t p s) c -> p t s c", p=128, s=2
                        ),
                    )
                    xin = xbf_pool.tile(
                        [128, 8 * 128], MM_DT, name=f"xin_{b}_{ci}", tag="xin"
                    )
                    ce = CAST_PAT[cast_ctr[0] % len(CAST_PAT)]
                    cast_ctr[0] += 1
                    if ce == "g":
                        nc.gpsimd.tensor_copy(xin[:, :], xf[:, :])
                    elif ce == "s":
                        nc.scalar.copy(xin[:, :], xf[:, :])
                    else:
                        nc.vector.tensor_copy(xin[:, :], xf[:, :])

                pt8 = pt_pool.tile([128, 1024], MM_DT, name=f"pt8_{b}_{ci}", tag="pt8")
                for j in range(T_IN):
                    nc.tensor.transpose(
                        pt8[:, j * 128 : (j + 1) * 128],
                        xin[:, j * 128 : (j + 1) * 128],
                        ident,
                    )
                q = xoff + ci * (IN_CHUNK // 2)
                nc.vector.tensor_copy(xP[0:C, q : q + 1024], pt8[0:C, :])
                nc.vector.tensor_copy(xP[C:128, q : q + 1024], pt8[C:128, :])

            def store_chunk(b, s):
                xoff = (b % 2) * XP_W
                o0 = s * 2 * ST_PAIRS
                osb = osb_pool.tile(
                    [128, 2 * ST_PAIRS], OUT_DT, name=f"osb_{b}_{s}", tag="osb"
                )
                for bk in range(4):
                    po = po_pool.tile(
                        [128, 512], fp32, name=f"po_{b}_{s}_{bk}", tag="po"
                    )
                    for pt in range(2):
                        j0 = xoff + s * ST_PAIRS + bk * BANK_PAIRS + pt * 128
                        c = pt * 256
                        # even: out[2j] = x[2j]w0 + x[2j+1]w1 + x[2j+2]w2
                        nc.tensor.matmul(
                            po[:, c : c + 128],
                            xP[:, j0 : j0 + 128],
                            wAll[:, 0:F],
                            start=True,
                            stop=False,
                        )
                        nc.tensor.matmul(
                            po[:, c : c + 128],
                            xP[:, j0 + 1 : j0 + 129],
                            wAll[:, 2 * F : 3 * F],
                            start=False,
                            stop=not has_bias,
                        )
                        if has_bias:
                            nc.tensor.matmul(
                                po[:, c : c + 128],
                                ones[:, :],
                                brow[:, :],
                                start=False,
                                stop=True,
                            )
                        # odd: out[2j+1] = x[2j+1]w0 + x[2j+2]w1 + x[2j+3]w2
                        nc.tensor.matmul(
                            po[:, c + 128 : c + 256],
                            xP[:, j0 : j0 + 128],
                            wAll[:, 3 * F : 4 * F],
                            start=True,
                            stop=False,
                        )
                        nc.tensor.matmul(
                            po[:, c + 128 : c + 256],
                            xP[:, j0 + 1 : j0 + 129],
                            wAll[:, F : 2 * F],
                            start=False,
                            stop=not has_bias,
                        )
                        if has_bias:
                            nc.tensor.matmul(
                                po[:, c + 128 : c + 256],
                                ones[:, :],
                                brow[:, :],
                                start=False,
                                stop=True,
                            )
                    ob = bk * 512
                    # ReLU + fp32->bf16, split 5:3 scalar:DVE
                    if relu_ctr[0] % 8 < 5:
                        nc.scalar.activation(
                            osb[:, ob : ob + 512],
                            po[:, :],
                            mybir.ActivationFunctionType.Relu,
                        )
                    else:
                        nc.vector.tensor_scalar_max(
                            osb[:, ob : ob + 512], po[:, :], 0.0
                        )
                    relu_ctr[0] += 1
                # stores: 512B descriptors (pair of positions), alternate rings
                eng = nc.scalar if store_ctr[0] % 2 == 0 else nc.sync
                store_ctr[0] += 1
                if s < N_ST - 1:
                    eng.dma_start(
                        out=out_ap[b, o0 : o0 + 2048, :].rearrange(
                            "(g p s2) f -> p g s2 f", p=128, s2=2
                        ),
                        in_=osb.rearrange("p (g s2 f) -> p g s2 f", s2=2, f=F),
                    )
                else:
                    # last store: drop garbage pair 4095 (positions 8190/8191)
                    eng.dma_start(
                        out=out_ap[b, o0 : o0 + 1792, :].rearrange(
                            "(g p s2) f -> p g s2 f", p=128, s2=2
                        ),
                        in_=osb[:, 0:1792].rearrange(
                            "p (g s2 f) -> p g s2 f", s2=2, f=F
                        ),
                    )
                    eng.dma_start(
                        out=out_ap[b, o0 + 1792 : o0 + 2046, :].rearrange(
                            "(p s2) f -> p s2 f", p=127, s2=2
                        ),
                        in_=osb[0:127, 1792:2048].rearrange(
                            "p (s2 f) -> p s2 f", s2=2
                        ),
                    )

            # Software pipeline with a 2-fill lookahead: store (b,s) needs
            # fill (b,s+1) (the +1 window crosses one col into chunk s+1;
            # s=3 needs only fill 3 + the zero pad), which this order always
            # satisfies. Interleaving fills of batch b+1 between the tail
            # stores of batch b keeps the input stream running across batch
            # boundaries (xP halves alternate, so no conflict).
            fills = [(b, ci) for b in range(B_SHARD) for ci in range(N_CI)]
            stores = [(b, s) for b in range(B_SHARD) for s in range(N_ST)]
            for f in fills[:2]:
                fill_chunk(*f)
            fi = 2
            for st in stores:
                if fi < len(fills):
                    fill_chunk(*fills[fi])
                    fi += 1
                store_chunk(*st)


def build_program(has_bias):
    nc = bacc.Bacc("TRN2", target_bir_lowering=False, debug=False)
    x = nc.dram_tensor("x", [B_SHARD, L, C], mybir.dt.float32, kind="ExternalInput")
    w = nc.dram_tensor("w", [K, C, F], mybir.dt.float32, kind="ExternalInput")
    bb = nc.dram_tensor("b", [F], mybir.dt.float32, kind="ExternalInput")
    out = nc.dram_tensor(
        "out", [B_SHARD, L_OUT, F], OUT_DT, kind="ExternalOutput"
    )
    with tile.TileContext(nc) as tc:
        _conv_kernel(tc, out.ap(), x.ap(), w.ap(), bb.ap(), has_bias)
    nc.compile()
    return nc


def kernel(x, w, b, _trace=False, _trace_kwargs=None):
    x = np.ascontiguousarray(np.asarray(x, dtype=np.float32))
    w = np.ascontiguousarray(np.asarray(w, dtype=np.float32))
    b = np.ascontiguousarray(np.asarray(b, dtype=np.float32))
    assert x.shape == (B, L, C) and w.shape == (K, C, F) and b.shape == (F,)

    nc = build_program(has_bias=bool(np.any(b)))
    in_maps = [
        {"x": x[i * B_SHARD : (i + 1) * B_SHARD], "w": w, "b": b}
        for i in range(N_CORES)
    ]
    res = run_bass_kernel_spmd(
        nc,
        in_maps,
        core_ids=list(range(N_CORES)),
        trace=_trace,
        **(_trace_kwargs or {}),
    )
    out = np.concatenate(
        [np.asarray(r["out"]).astype(np.float32) for r in res.results], axis=0
    )
    if _trace:
        return out, res
    return out


if __name__ == "__main__":
    rng = np.random.default_rng(0)
    x = rng.standard_normal((B, L, C), dtype=np.float32)
    w = rng.standard_normal((K, C, F), dtype=np.float32) * 0.08
    b = np.zeros((F,), dtype=np.float32)
    out = kernel(x, w, b)
    print("out", out.shape, out.dtype, float(np.abs(out).max()))


# revision 36
# speedup vs baseline: 1.6673x; 1.2603x over previous
"""Conv1D (B=32, L=8192, C_in=64, K=3, F=128, VALID) + bias + ReLU on 8 trn2 cores.

Data-parallel over batch (4 batches per core). Quad-polyphase design:

  - Input loads 4 consecutive positions per partition (1KB descriptors) on
    the sync HWDGE ring; fp32->bf16 casts alternate scalar/DVE (gpsimd casts
    are 4x slower and stall the pipeline; swDGE cast-DMA costs 42ns/256B
    descriptor vs 29ns/512B hwDGE, so neither is used).
  - PE-transpose of each [128,(s,c)] sub-tile yields phases (0,1) or (2,3)
    stacked in partitions; strided-source full-128-partition DVE copies
    build xE[:,q]=[x[4q];x[4q+1]] and xO[:,q]=[x[4q+2];x[4q+3]].
  - 8 c=128 matmuls per 128-quad PSUM bank (2 per output phase; c=64
    matmuls hard-crash the device so single-tap matmuls pad the weight tile
    with zero rows):
      pos 4q   = xE[q]@[w0;w1] + xO[q]@[w2;0]
      pos 4q+1 = xE[q]@[0;w0]  + xO[q]@[w1;w2]
      pos 4q+2 = xO[q]@[w0;w1] + xE[q+1]@[w2;0]
      pos 4q+3 = xO[q]@[0;w0]  + xE[q+1]@[w1;w2]
  - PSUM bank [128,512] = 128 quads x (4 phases x F); ReLU (split
    scalar/DVE) writes bf16 staging; partition q holds positions 4q..4q+3
    so stores are 1KB descriptors; bf16 output halves write bytes (host
    upcasts to fp32; rel err ~4e-3 << 2e-2 budget).
  - Global software pipeline: fills run 2 chunks ahead of stores and
    interleave across batch boundaries (xE/xO halves alternate per batch).
"""

import os
import sys

import numpy as np

_TRN_REPO = "/opt/trn_rl_repo"
if _TRN_REPO not in sys.path and os.path.isdir(_TRN_REPO):
    sys.path.insert(0, _TRN_REPO)

import concourse.bass as bass
import concourse.tile as tile
from concourse import bacc, mybir
from concourse.bass_utils import run_bass_kernel_spmd
from concourse.masks import make_identity

B, L, C = 32, 8192, 64
K, F = 3, 128
L_OUT = L - K + 1  # 8190
N_CORES = 8
B_SHARD = B // N_CORES  # 4

MM_DT = mybir.dt.bfloat16
OUT_DT = mybir.dt.bfloat16

IN_CHUNK = 2048  # positions per input DMA chunk (512 quads)
N_CI = L // IN_CHUNK  # 4
QUADS = L // 4  # 2048 quads per batch (quad 2047 phases 2,3 are garbage)
XQ_PAD = 8
XQ_W = QUADS + XQ_PAD  # 2056
ST_QUADS = 512  # quads per output staging tile (4 banks of 128)
N_ST = QUADS // ST_QUADS  # 4 stores per batch


def _conv_kernel(tc: tile.TileContext, out_ap, x_ap, w_ap, b_ap, has_bias):
    nc = tc.nc
    fp32 = mybir.dt.float32

    with (
        tc.tile_pool(name="setup", bufs=1) as setup_pool,
        tc.tile_pool(name="xin", bufs=4) as xin_pool,
        tc.tile_pool(name="xbf", bufs=4) as xbf_pool,
        tc.tile_pool(name="osb", bufs=4) as osb_pool,
        tc.tile_pool(name="pt", bufs=3, space="PSUM") as pt_pool,
        tc.tile_pool(name="po", bufs=4, space="PSUM") as po_pool,
    ):
        # prefetch the first input chunks before any setup work so the DMA
        # engines start streaming during the weights/identity preamble
        prefetched = {}
        for pb, pci in ((0, 0), (0, 1)):
            xf = xin_pool.tile([128, 1024], fp32, name=f"xf_{pb}_{pci}", tag="xf")
            nc.sync.dma_start(
                out=xf.rearrange("p (t s c) -> p t s c", s=4, c=C),
                in_=x_ap[pb, pci * IN_CHUNK : (pci + 1) * IN_CHUNK, :].rearrange(
                    "(t p s) c -> p t s c", p=128, s=4
                ),
            )
            prefetched[(pb, pci)] = xf

        # --- one-time setup ---
        wstage = setup_pool.tile([C, K * F], fp32)
        for k in range(K):
            nc.scalar.dma_start(out=wstage[:, k * F : (k + 1) * F], in_=w_ap[k])

        # wAll cols: [0:F]=[w0;w1]  [F:2F]=[w1;w2]  [2F:3F]=[w2;0]  [3F:4F]=[0;w0]
        wAll = setup_pool.tile([128, 4 * F], MM_DT)
        nc.vector.memset(wAll[:, :], 0.0)
        nc.vector.tensor_copy(wAll[0:C, 0:F], wstage[:, 0:F])
        nc.vector.tensor_copy(wAll[C:128, 0:F], wstage[:, F : 2 * F])
        nc.vector.tensor_copy(wAll[0:C, F : 2 * F], wstage[:, F : 2 * F])
        nc.vector.tensor_copy(wAll[C:128, F : 2 * F], wstage[:, 2 * F : 3 * F])
        nc.vector.tensor_copy(wAll[0:C, 2 * F : 3 * F], wstage[:, 2 * F : 3 * F])
        nc.vector.tensor_copy(wAll[C:128, 3 * F : 4 * F], wstage[:, 0:F])

        if has_bias:
            bstage = setup_pool.tile([1, F], fp32)
            nc.scalar.dma_start(out=bstage[:, :], in_=b_ap[None, :])
            ones = setup_pool.tile([1, 128], MM_DT)
            nc.vector.memset(ones[:, :], 1.0)
            brow = setup_pool.tile([1, F], MM_DT)
            nc.vector.tensor_copy(brow[:, :], bstage[:, :])

        ident = setup_pool.tile([128, 128], MM_DT)
        make_identity(nc, ident)

        # xE/xO: manually double-buffered packed pair arrays.
        xE = setup_pool.tile([128, 2 * XQ_W], MM_DT)
        xO = setup_pool.tile([128, 2 * XQ_W], MM_DT)
        for h in range(2):
            nc.vector.memset(xE[:, h * XQ_W + QUADS : (h + 1) * XQ_W], 0.0)
            nc.vector.memset(xO[:, h * XQ_W + QUADS : (h + 1) * XQ_W], 0.0)

        relu_ctr = [0]
        store_ctr = [0]
        cast_ctr = [0]

        def fill_chunk(b, ci):
            xoff = (b % 2) * XQ_W
            c0 = ci * IN_CHUNK
            if (b, ci) in prefetched:
                xf = prefetched.pop((b, ci))
            else:
                xf = xin_pool.tile([128, 1024], fp32, name=f"xf_{b}_{ci}", tag="xf")
                nc.sync.dma_start(
                    out=xf.rearrange("p (t s c) -> p t s c", s=4, c=C),
                    in_=x_ap[b, c0 : c0 + IN_CHUNK, :].rearrange(
                        "(t p s) c -> p t s c", p=128, s=4
                    ),
                )
            xin = xbf_pool.tile([128, 1024], MM_DT, name=f"xin_{b}_{ci}", tag="xin")
            if cast_ctr[0] % 2 == 0:
                nc.scalar.copy(xin[:, :], xf[:, :])
            else:
                nc.vector.tensor_copy(xin[:, :], xf[:, :])
            cast_ctr[0] += 1

            pt8 = pt_pool.tile([128, 1024], MM_DT, name=f"pt8_{b}_{ci}", tag="pt8")
            for j in range(8):
                nc.tensor.transpose(
                    pt8[:, j * 128 : (j + 1) * 128],
                    xin[:, j * 128 : (j + 1) * 128],
                    ident,
                )
            # even sub-tiles hold [ph0;ph1] -> xE, odd hold [ph2;ph3] -> xO
            q = xoff + ci * (IN_CHUNK // 4)
            src = pt8.rearrange("p (t par h) -> p t par h", par=2, h=128)
            nc.vector.tensor_copy(
                xE[:, q : q + 512].rearrange("p (t h) -> p t h", h=128),
                src[:, :, 0, :],
            )
            nc.vector.tensor_copy(
                xO[:, q : q + 512].rearrange("p (t h) -> p t h", h=128),
                src[:, :, 1, :],
            )

        def store_chunk(b, s):
            xoff = (b % 2) * XQ_W
            o0 = s * 4 * ST_QUADS
            osb = osb_pool.tile(
                [128, 4 * ST_QUADS], OUT_DT, name=f"osb_{b}_{s}", tag="osb"
            )
            for bk in range(4):
                po = po_pool.tile([128, 512], fp32, name=f"po_{b}_{s}_{bk}", tag="po")
                q0 = xoff + s * ST_QUADS + bk * 128
                for ph in range(4):
                    c = ph * 128
                    lhs1 = xE[:, q0 : q0 + 128] if ph < 2 else xO[:, q0 : q0 + 128]
                    lhs2 = (
                        xO[:, q0 : q0 + 128]
                        if ph < 2
                        else xE[:, q0 + 1 : q0 + 129]
                    )
                    w1c = (0 if ph % 2 == 0 else 3) * F
                    w2c = (2 if ph % 2 == 0 else 1) * F
                    nc.tensor.matmul(
                        po[:, c : c + 128],
                        lhs1,
                        wAll[:, w1c : w1c + F],
                        start=True,
                        stop=False,
                    )
                    nc.tensor.matmul(
                        po[:, c : c + 128],
                        lhs2,
                        wAll[:, w2c : w2c + F],
                        start=False,
                        stop=not has_bias,
                    )
                    if has_bias:
                        nc.tensor.matmul(
                            po[:, c : c + 128],
                            ones[:, :],
                            brow[:, :],
                            start=False,
                            stop=True,
                        )
                ob = bk * 512
                # ReLU + fp32->bf16, split 5:3 scalar:DVE
                if relu_ctr[0] % 8 < 5:
                    nc.scalar.activation(
                        osb[:, ob : ob + 512],
                        po[:, :],
                        mybir.ActivationFunctionType.Relu,
                    )
                else:
                    nc.vector.tensor_scalar_max(osb[:, ob : ob + 512], po[:, :], 0.0)
                relu_ctr[0] += 1
                # stores: 1KB descriptors (4 consecutive positions); issue per
                # half-osb so the final drain is short, alternating rings. The
                # out tensor is padded to L=8192 so every store is a uniform
                # 256-descriptor DMA — small (<128-desc) tail DMAs land
                # entirely on a single DMA engine and serialize it. Positions
                # 8190/8191 hold relu(pad)=0 garbage; the host slices them off.
                if bk % 2 == 1:
                    g0 = bk - 1
                    eng = nc.scalar if store_ctr[0] % 2 == 0 else nc.sync
                    store_ctr[0] += 1
                    eng.dma_start(
                        out=out_ap[
                            b, o0 + g0 * 512 : o0 + (g0 + 2) * 512, :
                        ].rearrange("(g p s4) f -> p g s4 f", p=128, s4=4),
                        in_=osb[:, g0 * 512 : (g0 + 2) * 512].rearrange(
                            "p (g s4 f) -> p g s4 f", s4=4, f=F
                        ),
                    )

        # Software pipeline with a 2-fill lookahead: store (b,s) needs fill
        # (b,s+1) (the +1 window crosses one col into chunk s+1; s=3 needs
        # only fill 3 + the zero pad), which this order always satisfies.
        fills = [(b, ci) for b in range(B_SHARD) for ci in range(N_CI)]
        stores = [(b, s) for b in range(B_SHARD) for s in range(N_ST)]
        for f in fills[:3]:
            fill_chunk(*f)
        fi = 3
        for st in stores:
            if fi < len(fills):
                fill_chunk(*fills[fi])
                fi += 1
            store_chunk(*st)


def build_program(has_bias):
    nc = bacc.Bacc("TRN2", target_bir_lowering=False, debug=False)
    x = nc.dram_tensor("x", [B_SHARD, L, C], mybir.dt.float32, kind="ExternalInput")
    w = nc.dram_tensor("w", [K, C, F], mybir.dt.float32, kind="ExternalInput")
    bb = nc.dram_tensor("b", [F], mybir.dt.float32, kind="ExternalInput")
    out = nc.dram_tensor("out", [B_SHARD, L, F], OUT_DT, kind="ExternalOutput")
    with tile.TileContext(nc) as tc:
        _conv_kernel(tc, out.ap(), x.ap(), w.ap(), bb.ap(), has_bias)
    nc.compile()
    return nc


def kernel(x, w, b, _trace=False, _trace_kwargs=None):
    x = np.ascontiguousarray(np.asarray(x, dtype=np.float32))
    w = np.ascontiguousarray(np.asarray(w, dtype=np.float32))
    b = np.ascontiguousarray(np.asarray(b, dtype=np.float32))
    assert x.shape == (B, L, C) and w.shape == (K, C, F) and b.shape == (F,)

    nc = build_program(has_bias=bool(np.any(b)))
    in_maps = [
        {"x": x[i * B_SHARD : (i + 1) * B_SHARD], "w": w, "b": b}
        for i in range(N_CORES)
    ]
    res = run_bass_kernel_spmd(
        nc,
        in_maps,
        core_ids=list(range(N_CORES)),
        trace=_trace,
        **(_trace_kwargs or {}),
    )
    out = np.concatenate(
        [np.asarray(r["out"])[:, :L_OUT, :].astype(np.float32) for r in res.results],
        axis=0,
    )
    if _trace:
        return out, res
    return out


if __name__ == "__main__":
    rng = np.random.default_rng(0)
    x = rng.standard_normal((B, L, C), dtype=np.float32)
    w = rng.standard_normal((K, C, F), dtype=np.float32) * 0.08
    b = np.zeros((F,), dtype=np.float32)
    out = kernel(x, w, b)
    print("out", out.shape, out.dtype, float(np.abs(out).max()))


# revision 37
# speedup vs baseline: 1.7381x; 1.0425x over previous
"""Conv1D (B=32, L=8192, C_in=64, K=3, F=128, VALID) + bias + ReLU on 8 trn2 cores.

Data-parallel over batch (4 batches per core). Quad-polyphase design:

  - Input loads 4 consecutive positions per partition (1KB descriptors) on
    the sync HWDGE ring; fp32->bf16 casts alternate scalar/DVE (gpsimd casts
    are 4x slower and stall the pipeline; swDGE cast-DMA costs 42ns/256B
    descriptor vs 29ns/512B hwDGE, so neither is used).
  - PE-transpose of each [128,(s,c)] sub-tile yields phases (0,1) or (2,3)
    stacked in partitions; strided-source full-128-partition DVE copies
    build xE[:,q]=[x[4q];x[4q+1]] and xO[:,q]=[x[4q+2];x[4q+3]].
  - 8 c=128 matmuls per 128-quad PSUM bank (2 per output phase; c=64
    matmuls hard-crash the device so single-tap matmuls pad the weight tile
    with zero rows):
      pos 4q   = xE[q]@[w0;w1] + xO[q]@[w2;0]
      pos 4q+1 = xE[q]@[0;w0]  + xO[q]@[w1;w2]
      pos 4q+2 = xO[q]@[w0;w1] + xE[q+1]@[w2;0]
      pos 4q+3 = xO[q]@[0;w0]  + xE[q+1]@[w1;w2]
  - PSUM bank [128,512] = 128 quads x (4 phases x F); ReLU (split
    scalar/DVE) writes bf16 staging; partition q holds positions 4q..4q+3
    so stores are 1KB descriptors; bf16 output halves write bytes (host
    upcasts to fp32; rel err ~4e-3 << 2e-2 budget).
  - Global software pipeline: fills run 2 chunks ahead of stores and
    interleave across batch boundaries (xE/xO halves alternate per batch).
"""

import os
import sys

import numpy as np

_TRN_REPO = "/opt/trn_rl_repo"
if _TRN_REPO not in sys.path and os.path.isdir(_TRN_REPO):
    sys.path.insert(0, _TRN_REPO)

import concourse.bass as bass
import concourse.tile as tile
from concourse import bacc, mybir
from concourse.bass_utils import run_bass_kernel_spmd
from concourse.masks import make_identity

B, L, C = 32, 8192, 64
K, F = 3, 128
L_OUT = L - K + 1  # 8190
N_CORES = 8
B_SHARD = B // N_CORES  # 4

MM_DT = mybir.dt.bfloat16
OUT_DT = mybir.dt.bfloat16

IN_CHUNK = 2048  # positions per input DMA chunk (512 quads)
N_CI = L // IN_CHUNK  # 4
QUADS = L // 4  # 2048 quads per batch (quad 2047 phases 2,3 are garbage)
XQ_PAD = 8
XQ_W = QUADS + XQ_PAD  # 2056
ST_QUADS = 512  # quads per output staging tile (4 banks of 128)
N_ST = QUADS // ST_QUADS  # 4 stores per batch


def _conv_kernel(tc: tile.TileContext, out_ap, x_ap, w_ap, b_ap, has_bias):
    nc = tc.nc
    fp32 = mybir.dt.float32

    with (
        tc.tile_pool(name="setup", bufs=1) as setup_pool,
        tc.tile_pool(name="xin", bufs=4) as xin_pool,
        tc.tile_pool(name="xbf", bufs=4) as xbf_pool,
        tc.tile_pool(name="osb", bufs=4) as osb_pool,
        tc.tile_pool(name="pt", bufs=3, space="PSUM") as pt_pool,
        tc.tile_pool(name="po", bufs=4, space="PSUM") as po_pool,
    ):
        # prefetch the first input chunks before any setup work so the DMA
        # engines start streaming during the weights/identity preamble
        prefetched = {}
        for pb, pci in ((0, 0), (0, 1)):
            xf = xin_pool.tile([128, 1024], fp32, name=f"xf_{pb}_{pci}", tag="xf")
            nc.sync.dma_start(
                out=xf.rearrange("p (t s c) -> p t s c", s=4, c=C),
                in_=x_ap[pb, pci * IN_CHUNK : (pci + 1) * IN_CHUNK, :].rearrange(
                    "(t p s) c -> p t s c", p=128, s=4
                ),
            )
            prefetched[(pb, pci)] = xf

        # --- one-time setup ---
        wstage = setup_pool.tile([C, K * F], fp32)
        for k in range(K):
            nc.scalar.dma_start(out=wstage[:, k * F : (k + 1) * F], in_=w_ap[k])

        # wAll cols: [0:F]=[w0;w1]  [F:2F]=[w1;w2]  [2F:3F]=[w2;0]  [3F:4F]=[0;w0]
        wAll = setup_pool.tile([128, 4 * F], MM_DT)
        nc.vector.memset(wAll[:, :], 0.0)
        nc.vector.tensor_copy(wAll[0:C, 0:F], wstage[:, 0:F])
        nc.vector.tensor_copy(wAll[C:128, 0:F], wstage[:, F : 2 * F])
        nc.vector.tensor_copy(wAll[0:C, F : 2 * F], wstage[:, F : 2 * F])
        nc.vector.tensor_copy(wAll[C:128, F : 2 * F], wstage[:, 2 * F : 3 * F])
        nc.vector.tensor_copy(wAll[0:C, 2 * F : 3 * F], wstage[:, 2 * F : 3 * F])
        nc.vector.tensor_copy(wAll[C:128, 3 * F : 4 * F], wstage[:, 0:F])

        if has_bias:
            bstage = setup_pool.tile([1, F], fp32)
            nc.scalar.dma_start(out=bstage[:, :], in_=b_ap[None, :])
            ones = setup_pool.tile([1, 128], MM_DT)
            nc.vector.memset(ones[:, :], 1.0)
            brow = setup_pool.tile([1, F], MM_DT)
            nc.vector.tensor_copy(brow[:, :], bstage[:, :])

        ident = setup_pool.tile([128, 128], MM_DT)
        make_identity(nc, ident)

        # xE/xO: manually double-buffered packed pair arrays.
        xE = setup_pool.tile([128, 2 * XQ_W], MM_DT)
        xO = setup_pool.tile([128, 2 * XQ_W], MM_DT)
        for h in range(2):
            nc.vector.memset(xE[:, h * XQ_W + QUADS : (h + 1) * XQ_W], 0.0)
            nc.vector.memset(xO[:, h * XQ_W + QUADS : (h + 1) * XQ_W], 0.0)

        relu_ctr = [0]
        store_ctr = [0]
        cast_ctr = [0]

        def fill_chunk(b, ci):
            xoff = (b % 2) * XQ_W
            c0 = ci * IN_CHUNK
            if (b, ci) in prefetched:
                xf = prefetched.pop((b, ci))
            else:
                xf = xin_pool.tile([128, 1024], fp32, name=f"xf_{b}_{ci}", tag="xf")
                nc.sync.dma_start(
                    out=xf.rearrange("p (t s c) -> p t s c", s=4, c=C),
                    in_=x_ap[b, c0 : c0 + IN_CHUNK, :].rearrange(
                        "(t p s) c -> p t s c", p=128, s=4
                    ),
                )
            xin = xbf_pool.tile([128, 1024], MM_DT, name=f"xin_{b}_{ci}", tag="xin")
            if cast_ctr[0] % 2 == 0:
                nc.scalar.copy(xin[:, :], xf[:, :])
            else:
                nc.vector.tensor_copy(xin[:, :], xf[:, :])
            cast_ctr[0] += 1

            pt8 = pt_pool.tile([128, 1024], MM_DT, name=f"pt8_{b}_{ci}", tag="pt8")
            for j in range(8):
                nc.tensor.transpose(
                    pt8[:, j * 128 : (j + 1) * 128],
                    xin[:, j * 128 : (j + 1) * 128],
                    ident,
                )
            # even sub-tiles hold [ph0;ph1] -> xE, odd hold [ph2;ph3] -> xO
            q = xoff + ci * (IN_CHUNK // 4)
            src = pt8.rearrange("p (t par h) -> p t par h", par=2, h=128)
            nc.vector.tensor_copy(
                xE[:, q : q + 512].rearrange("p (t h) -> p t h", h=128),
                src[:, :, 0, :],
            )
            nc.vector.tensor_copy(
                xO[:, q : q + 512].rearrange("p (t h) -> p t h", h=128),
                src[:, :, 1, :],
            )

        def store_chunk(b, s):
            xoff = (b % 2) * XQ_W
            o0 = s * 4 * ST_QUADS
            osb = osb_pool.tile(
                [128, 4 * ST_QUADS], OUT_DT, name=f"osb_{b}_{s}", tag="osb"
            )
            for bk in range(4):
                po = po_pool.tile([128, 512], fp32, name=f"po_{b}_{s}_{bk}", tag="po")
                q0 = xoff + s * ST_QUADS + bk * 128
                for ph in range(4):
                    c = ph * 128
                    lhs1 = xE[:, q0 : q0 + 128] if ph < 2 else xO[:, q0 : q0 + 128]
                    lhs2 = (
                        xO[:, q0 : q0 + 128]
                        if ph < 2
                        else xE[:, q0 + 1 : q0 + 129]
                    )
                    w1c = (0 if ph % 2 == 0 else 3) * F
                    w2c = (2 if ph % 2 == 0 else 1) * F
                    nc.tensor.matmul(
                        po[:, c : c + 128],
                        lhs1,
                        wAll[:, w1c : w1c + F],
                        start=True,
                        stop=False,
                    )
                    nc.tensor.matmul(
                        po[:, c : c + 128],
                        lhs2,
                        wAll[:, w2c : w2c + F],
                        start=False,
                        stop=not has_bias,
                    )
                    if has_bias:
                        nc.tensor.matmul(
                            po[:, c : c + 128],
                            ones[:, :],
                            brow[:, :],
                            start=False,
                            stop=True,
                        )
                ob = bk * 512
                # ReLU + fp32->bf16, split 5:3 scalar:DVE
                if relu_ctr[0] % 8 < 5:
                    nc.scalar.activation(
                        osb[:, ob : ob + 512],
                        po[:, :],
                        mybir.ActivationFunctionType.Relu,
                    )
                else:
                    nc.vector.tensor_scalar_max(osb[:, ob : ob + 512], po[:, :], 0.0)
                relu_ctr[0] += 1
                # stores: 1KB descriptors (4 consecutive positions); issue per
                # half-osb so the final drain is short, alternating rings. The
                # out tensor is padded to L=8192 so every store is a uniform
                # 256-descriptor DMA — small (<128-desc) tail DMAs land
                # entirely on a single DMA engine and serialize it. Positions
                # 8190/8191 hold relu(pad)=0 garbage; the host slices them off.
                if bk % 2 == 1:
                    g0 = bk - 1
                    eng = nc.scalar if store_ctr[0] % 2 == 0 else nc.sync
                    store_ctr[0] += 1
                    eng.dma_start(
                        out=out_ap[
                            b, o0 + g0 * 512 : o0 + (g0 + 2) * 512, :
                        ].rearrange("(g p s4) f -> p g s4 f", p=128, s4=4),
                        in_=osb[:, g0 * 512 : (g0 + 2) * 512].rearrange(
                            "p (g s4 f) -> p g s4 f", s4=4, f=F
                        ),
                    )

        # Software pipeline with a 2-fill lookahead: store (b,s) needs fill
        # (b,s+1) (the +1 window crosses one col into chunk s+1; s=3 needs
        # only fill 3 + the zero pad), which this order always satisfies.
        fills = [(b, ci) for b in range(B_SHARD) for ci in range(N_CI)]
        stores = [(b, s) for b in range(B_SHARD) for s in range(N_ST)]
        for f in fills[:2]:
            fill_chunk(*f)
        fi = 2
        for st in stores:
            if fi < len(fills):
                fill_chunk(*fills[fi])
                fi += 1
            store_chunk(*st)


def build_program(has_bias):
    nc = bacc.Bacc("TRN2", target_bir_lowering=False, debug=False)
    x = nc.dram_tensor("x", [B_SHARD, L, C], mybir.dt.float32, kind="ExternalInput")
    w = nc.dram_tensor("w", [K, C, F], mybir.dt.float32, kind="ExternalInput")
    bb = nc.dram_tensor("b", [F], mybir.dt.float32, kind="ExternalInput")
    out = nc.dram_tensor("out", [B_SHARD, L, F], OUT_DT, kind="ExternalOutput")
    with tile.TileContext(nc) as tc:
        _conv_kernel(tc, out.ap(), x.ap(), w.ap(), bb.ap(), has_bias)
    nc.compile()
    return nc


def kernel(x, w, b, _trace=False, _trace_kwargs=None):
    x = np.ascontiguousarray(np.asarray(x, dtype=np.float32))
    w = np.ascontiguousarray(np.asarray(w, dtype=np.float32))
    b = np.ascontiguousarray(np.asarray(b, dtype=np.float32))
    assert x.shape == (B, L, C) and w.shape == (K, C, F) and b.shape == (F,)

    nc = build_program(has_bias=bool(np.any(b)))
    in_maps = [
        {"x": x[i * B_SHARD : (i + 1) * B_SHARD], "w": w, "b": b}
        for i in range(N_CORES)
    ]
    res = run_bass_kernel_spmd(
        nc,
        in_maps,
        core_ids=list(range(N_CORES)),
        trace=_trace,
        **(_trace_kwargs or {}),
    )
    out = np.concatenate(
        [np.asarray(r["out"])[:, :L_OUT, :].astype(np.float32) for r in res.results],
        axis=0,
    )
    if _trace:
        return out, res
    return out


if __name__ == "__main__":
    rng = np.random.default_rng(0)
    x = rng.standard_normal((B, L, C), dtype=np.float32)
    w = rng.standard_normal((K, C, F), dtype=np.float32) * 0.08
    b = np.zeros((F,), dtype=np.float32)
    out = kernel(x, w, b)
    print("out", out.shape, out.dtype, float(np.abs(out).max()))
